# revision 1
# baseline (speedup 1.0000x reference)
"""Trainium2 Bass kernel for nn_CrAKNVectorAttention (N=1024, C=256, 8 cores).

Math: the reference computes
    w   = softmax(h, axis=-2)                  # over j
    out[i,k] = (sum_j w[i,j,k]) * v[i,k]
and sum_j softmax_j(...) == 1 exactly, so the whole [N,C,C] relation cube
(q/k projections, LayerNorms, Mish, weight_encoding MLP) cancels out:
    out = v = feat @ Wv + bv
(verified numerically: ~4.5e-7 relative deviation, pure fp32 rounding in
the softmax normalization).

Sharding: data-parallel over N across 8 cores (128 rows each); Wv/bv
replicated.  Per core the kernel computes outT = Wv.T @ featT + bv in two
output-channel chunks:

  - Inputs arrive as one packed [128, 770] buffer sliced by FOUR parallel
    DMAs — three on SP (HWDGE) + one on Pool (SWDGE) — hoisted ahead of the
    Bass preamble barrier so the first transfer starts at t=0.  Slices are
    ordered so the PE never stalls: each of the 4 matmuls' operands land
    just before it is issued.
  - 4 matmuls (fp32, K=128 chunks) accumulate into two PSUM banks.
  - DVE evicts each PSUM chunk with a fused per-partition bias add
    (tensor_scalar_add).
  - Output ships via two pre-armed SWDGE scatters (descriptors generated
    early on Pool with iota-built indices; a cheap trigger_dma fires each
    as its eviction lands) — skipping the HWDGE + DGE-delay latency of a
    normal store DMA.  The scatter adds into the runtime-pre-zeroed output
    buffer, padded to 384 rows so the index tensor can span all 128
    partitions (only partitions 0..15 are semantically read; the host
    discards rows 256..383).

Degrades through a 3-tier ladder if the aggressive machinery fails in
the target environment: fast (above) -> mid (same hoisted input, plain
HWDGE output DMA) -> conservative TileContext version.
"""

import numpy as np

N, C = 1024, 256
N_CORES = 8
ROWS = N // N_CORES  # 128
P = 128
W_PK = 770

_CACHE = {}


def _build_fast():
    import concourse.bacc as bacc
    import concourse.mybir as mybir

    f32 = mybir.dt.float32
    i16 = mybir.dt.int16
    nc = bacc.Bacc("TRN2", target_bir_lowering=False, debug=False,
                   num_devices=N_CORES)

    pk_d = nc.dram_tensor("pk", [P, W_PK], f32, kind="ExternalInput").ap()
    # padded to 384 rows: iota-generated scatter indices from unused SBUF
    # partitions (16..127) land in rows [256:384), which the host discards
    out_d = nc.dram_tensor("outT", [C + P, ROWS], f32,
                           kind="ExternalOutput").ap()

    n_pre = len(nc.main_func.blocks[0].instructions)

    with (
        nc.sbuf_tensor([P, 256], f32) as A_t,
        nc.sbuf_tensor([P, 128], f32) as B_t,
        nc.sbuf_tensor([P, 256], f32) as C_t,
        nc.sbuf_tensor([P, 130], f32) as D_t,
        nc.sbuf_tensor([P, 2 * ROWS], f32) as ot_t,
        nc.sbuf_tensor([P, 16], i16) as idx_t,
        nc.psum_tensor([P, 512], f32) as psb0,
        nc.psum_tensor([P, 512], f32) as psb1,
        nc.semaphore() as d1,
        nc.semaphore() as dp,
        nc.semaphore() as d2,
        nc.semaphore() as d3,
        nc.semaphore() as pes,
        nc.semaphore() as v0,
        nc.semaphore() as v1,
        nc.semaphore() as prep_sem,
        nc.semaphore() as dout,
    ):
        A = A_t.ap()
        Bt = B_t.ap()
        Ct = C_t.ap()
        D = D_t.ap()
        ot = ot_t.ap()
        idx = idx_t.ap()
        ps0 = psb0.ap()[:, 0:ROWS]
        ps1 = psb1.ap()[:, 0:ROWS]

        # input DMAs (hoisted to t=0)
        nc.sync.dma_start(A[:], pk_d[:, 0:256]).then_inc(d1, 16)
        nc.sync.dma_start(Ct[:], pk_d[:, 384:640]).then_inc(d2, 16)
        nc.sync.dma_start(D[:], pk_d[:, 640:770]).then_inc(d3, 16)
        nc.gpsimd.dma_start(Bt[:], pk_d[:, 256:384]).then_inc(dp, 16)

        # Pool: scatter indices + pre-armed output scatters
        nc.gpsimd.iota(idx[:, 0:8], [[16, 8]], base=0, channel_multiplier=1)
        nc.gpsimd.iota(idx[:, 8:16], [[16, 8]], base=128, channel_multiplier=1)
        nc.gpsimd.dma_scatter_add(
            out_d[:, :], ot[:, 0:ROWS].rearrange("p (g m) -> p g m", g=1),
            idx[:, 0:8], ROWS, ROWS, ROWS,
            prepare_only=True, sem=dout).then_inc(prep_sem, 1)
        nc.gpsimd.dma_scatter_add(
            out_d[:, :], ot[:, ROWS:2 * ROWS].rearrange("p (g m) -> p g m", g=1),
            idx[:, 8:16], ROWS, ROWS, ROWS,
            prepare_only=True, sem=dout).then_inc(prep_sem, 1)

        # PE: 4 matmuls in chunk-arrival order
        nc.tensor.wait_ge(d1, 16)
        nc.tensor.matmul(ps0, A[:, 128:256], A[:, 0:128], start=True, stop=False)
        nc.tensor.wait_ge(dp, 16)
        nc.tensor.matmul(ps1, Bt[:, 0:128], A[:, 0:128], start=True, stop=False)
        nc.tensor.wait_ge(d2, 16)
        nc.tensor.matmul(ps0, Ct[:, 128:256], Ct[:, 0:128],
                         start=False, stop=True).then_inc(pes, 1)
        nc.tensor.wait_ge(d3, 16)
        nc.tensor.matmul(ps1, D[:, 0:128], Ct[:, 0:128],
                         start=False, stop=True).then_inc(pes, 1)

        # DVE: per-chunk eviction with fused bias
        nc.vector.wait_ge(pes, 1)
        nc.vector.tensor_scalar_add(
            ot[:, 0:ROWS], ps0, D[:, 128:129]).then_inc(v0, 1)
        nc.vector.wait_ge(pes, 2)
        nc.vector.tensor_scalar_add(
            ot[:, ROWS:2 * ROWS], ps1,
            D[:, 129:130]).then_inc(v1, 1)

        # Pool: fire scatters as evictions land
        nc.gpsimd.wait_ge(prep_sem, 1)
        t0 = nc.gpsimd.trigger_dma(count=1)
        t0._wait_ge(v0, 1)
        nc.gpsimd.wait_ge(prep_sem, 2)
        t1 = nc.gpsimd.trigger_dma(count=1)
        t1._wait_ge(v1, 1)
        nc.sync.wait_ge(dout, 32)

        # hoist the input DMAs and index generation ahead of the Bass
        # preamble (const memsets + all-engine barrier): they touch only
        # our tiles, and the preamble barrier otherwise delays the first
        # transfer by ~650ns
        insts = nc.main_func.blocks[0].instructions
        moved = [i for i in insts[n_pre:]
                 if type(i).__name__ == "InstDMACopy"
                 and i.engine in (mybir.EngineType.SP,
                                  mybir.EngineType.Pool)][:4]
        moved += [i for i in insts[n_pre:]
                  if type(i).__name__ == "InstIota"][:2]
        for m in moved:
            insts.remove(m)
        for m in reversed(moved):
            insts.insert(0, m)

    nc.compile()
    return nc


def _build_mid():
    """Middle fallback: same 4-way hoisted input + raw semaphores, but a
    plain HWDGE output DMA instead of the prepared-scatter machinery."""
    import concourse.bacc as bacc
    import concourse.mybir as mybir

    f32 = mybir.dt.float32
    nc = bacc.Bacc("TRN2", target_bir_lowering=False, debug=False,
                   num_devices=N_CORES)

    pk_d = nc.dram_tensor("pk", [P, W_PK], f32, kind="ExternalInput").ap()
    out_d = nc.dram_tensor("outT", [C + P, ROWS], f32,
                           kind="ExternalOutput").ap()

    n_pre = len(nc.main_func.blocks[0].instructions)

    with (
        nc.sbuf_tensor([P, 256], f32) as A_t,
        nc.sbuf_tensor([P, 128], f32) as B_t,
        nc.sbuf_tensor([P, 256], f32) as C_t,
        nc.sbuf_tensor([P, 130], f32) as D_t,
        nc.sbuf_tensor([P, 2 * ROWS], f32) as ot_t,
        nc.psum_tensor([P, 512], f32) as psb0,
        nc.psum_tensor([P, 512], f32) as psb1,
        nc.semaphore() as d1,
        nc.semaphore() as dp,
        nc.semaphore() as d2,
        nc.semaphore() as d3,
        nc.semaphore() as pes,
        nc.semaphore() as v1,
        nc.semaphore() as dout,
    ):
        A = A_t.ap()
        Bt = B_t.ap()
        Ct = C_t.ap()
        D = D_t.ap()
        ot = ot_t.ap()
        ps0 = psb0.ap()[:, 0:ROWS]
        ps1 = psb1.ap()[:, 0:ROWS]

        nc.sync.dma_start(A[:], pk_d[:, 0:256]).then_inc(d1, 16)
        nc.sync.dma_start(Ct[:], pk_d[:, 384:640]).then_inc(d2, 16)
        nc.sync.dma_start(D[:], pk_d[:, 640:770]).then_inc(d3, 16)
        nc.gpsimd.dma_start(Bt[:], pk_d[:, 256:384]).then_inc(dp, 16)

        nc.tensor.wait_ge(d1, 16)
        nc.tensor.matmul(ps0, A[:, 128:256], A[:, 0:128], start=True, stop=False)
        nc.tensor.wait_ge(dp, 16)
        nc.tensor.matmul(ps1, Bt[:, 0:128], A[:, 0:128], start=True, stop=False)
        nc.tensor.wait_ge(d2, 16)
        nc.tensor.matmul(ps0, Ct[:, 128:256], Ct[:, 0:128],
                         start=False, stop=True).then_inc(pes, 1)
        nc.tensor.wait_ge(d3, 16)
        nc.tensor.matmul(ps1, D[:, 0:128], Ct[:, 0:128],
                         start=False, stop=True).then_inc(pes, 1)

        nc.vector.wait_ge(pes, 1)
        nc.vector.tensor_scalar_add(ot[:, 0:ROWS], ps0, D[:, 128:129])
        nc.vector.wait_ge(pes, 2)
        nc.vector.tensor_scalar_add(
            ot[:, ROWS:2 * ROWS], ps1, D[:, 129:130]).then_inc(v1, 1)

        nc.sync.wait_ge(v1, 1)
        nc.sync.dma_start(
            out_d[0:C].rearrange("(a p) m -> p a m", a=2),
            ot.rearrange("p (a m) -> p a m", a=2)).then_inc(dout, 16)
        nc.sync.wait_ge(dout, 16)

        insts = nc.main_func.blocks[0].instructions
        moved = [i for i in insts[n_pre:]
                 if type(i).__name__ == "InstDMACopy"
                 and i.engine in (mybir.EngineType.SP,
                                  mybir.EngineType.Pool)][:4]
        for m in moved:
            insts.remove(m)
        for m in reversed(moved):
            insts.insert(0, m)

    nc.compile()
    return nc


def _build_fallback():
    """Plain Tile version: 2-way split packed input, 4 matmuls, DVE
    bias-add eviction, single output DMA, transposed output layout."""
    import concourse.bacc as bacc
    import concourse.bass as bass
    import concourse.mybir as mybir
    from concourse import tile

    f32 = mybir.dt.float32
    nc = bacc.Bacc("TRN2", target_bir_lowering=False, debug=False,
                   num_devices=N_CORES)

    pk_d = nc.dram_tensor("pk", [P, W_PK], f32, kind="ExternalInput").ap()
    out_d = nc.dram_tensor("outT", [C + P, ROWS], f32,
                           kind="ExternalOutput").ap()

    with tile.TileContext(nc) as tc:
        with (
            tc.tile_pool(name="sbuf", bufs=1) as pool,
            tc.tile_pool(name="psum", bufs=1, space=bass.MemorySpace.PSUM) as pp,
        ):
            pkA = pool.tile([P, 384], f32)   # ftA | wvA_a0 | wvA_a1
            pkB = pool.tile([P, 386], f32)   # ftB | wvB_a0 | wvB_a1 | bias
            ps0 = pp.tile([P, ROWS], f32, name="ps0")
            ps1 = pp.tile([P, ROWS], f32, name="ps1")
            ot = pool.tile([P, 2 * ROWS], f32)

            nc.sync.dma_start(pkA[:], pk_d[:, 0:384])
            nc.sync.dma_start(pkB[:], pk_d[:, 384:770])

            nc.tensor.matmul(ps0[:], pkA[:, 128:256], pkA[:, 0:128],
                             start=True, stop=False)
            nc.tensor.matmul(ps1[:], pkA[:, 256:384], pkA[:, 0:128],
                             start=True, stop=False)
            nc.tensor.matmul(ps0[:], pkB[:, 128:256], pkB[:, 0:128],
                             start=False, stop=True)
            nc.tensor.matmul(ps1[:], pkB[:, 256:384], pkB[:, 0:128],
                             start=False, stop=True)

            nc.vector.tensor_scalar_add(ot[:, 0:ROWS], ps0[:],
                                        pkB[:, 384:385])
            nc.vector.tensor_scalar_add(ot[:, ROWS:2 * ROWS], ps1[:],
                                        pkB[:, 385:386])

            nc.sync.dma_start(
                out_d[0:C].rearrange("(a p) m -> p a m", a=2),
                ot.rearrange("p (a m) -> p a m", a=2))

    nc.compile()
    return nc


def pack_inputs(feat, Wv, bv):
    feat = np.asarray(feat, dtype=np.float32)
    Wv = np.ascontiguousarray(np.asarray(Wv, dtype=np.float32))
    bv = np.asarray(bv, dtype=np.float32).reshape(C)
    bt = bv.reshape(2, P).T  # [P, 2]; col a holds bv[a*128 + p]
    maps = []
    for c in range(N_CORES):
        ftT = feat[c * ROWS:(c + 1) * ROWS, :].T  # [C, ROWS]
        pk = np.empty((P, W_PK), np.float32)
        pk[:, 0:128] = ftT[0:P, :]            # ftA
        pk[:, 128:256] = Wv[0:P, 0:128]       # wvA_a0
        pk[:, 256:384] = Wv[0:P, 128:256]     # wvA_a1
        pk[:, 384:512] = ftT[P:C, :]          # ftB
        pk[:, 512:640] = Wv[P:C, 0:128]       # wvB_a0
        pk[:, 640:768] = Wv[P:C, 128:256]     # wvB_a1
        pk[:, 768:770] = bt                   # bias
        maps.append({"pk": pk})
    return maps


_BUILDERS = [_build_fast, _build_mid, _build_fallback]


def _get_nc():
    if "nc" not in _CACHE:
        last = None
        for i, build in enumerate(_BUILDERS[_CACHE.get("tier", 0):],
                                  start=_CACHE.get("tier", 0)):
            try:
                _CACHE["nc"] = build()
                _CACHE["tier"] = i
                break
            except Exception as e:
                last = e
        else:
            raise last
    return _CACHE["nc"]


def _run(inputs, **run_kwargs):
    from concourse.bass_utils import run_bass_kernel_spmd

    nc = _get_nc()
    in_maps = pack_inputs(inputs["feat"], inputs["Wv"], inputs["bv"])
    res = run_bass_kernel_spmd(nc, in_maps, list(range(N_CORES)), **run_kwargs)
    parts = [np.ascontiguousarray(res.results[c]["outT"][0:C].T)
             for c in range(N_CORES)]
    return np.concatenate(parts, axis=0), res


def kernel(**inputs) -> np.ndarray:
    while True:
        try:
            out, _ = _run(inputs)
            return out
        except Exception:
            # demote to the next, more conservative program tier and retry
            tier = _CACHE.get("tier", 0) + 1
            if "nc" not in _CACHE or tier >= len(_BUILDERS):
                raise
            _CACHE.pop("nc")
            _CACHE["tier"] = tier



# revision 2
# speedup vs baseline: 1.2689x; 1.2689x over previous
"""Trainium2 Bass kernel for nn_CrAKNVectorAttention (N=1024, C=256, 8 cores).

Math: the reference computes
    w   = softmax(h, axis=-2)                  # over j
    out[i,k] = (sum_j w[i,j,k]) * v[i,k]
and sum_j softmax_j(...) == 1 exactly, so the whole [N,C,C] relation cube
(q/k projections, LayerNorms, Mish, weight_encoding MLP) cancels out:
    out = v = feat @ Wv + bv
(verified numerically: ~4.5e-7 relative deviation in fp32).

Sharding: data-parallel over N across 8 cores (128 rows each); Wv/bv
replicated.

Fast tier (v3, bf16 datapath, out^T layout, ~4.6us device occupancy):
  - chunk1 [ft(k 0:128) | Wv[0:128,:] | bias cols] via one SP HWDGE DMA --
    its transfer starts at the 1.3us HWDGE floor.
  - chunk2 [ft(k 128:256) | Wv[128:256,:]] via a pre-armed Pool SWDGE
    gather; the trigger fires the armed descriptors straight into the DMA
    engines (no HWDGE stage, no DGE delay), so its transfer queues
    immediately behind chunk1's.  The gather ucode consumes the idx stream
    one 16-entry vector ahead of the nominal wrapped layout, so the idx
    iota is baked with base=-16 over 9 columns to compensate.
  - PE computes out^T in two psum banks (psA = ch 0:128, psB = ch
    128:256), two bf16 matmuls per bank (contraction split k=2x128).
  - Evictions fuse the per-partition bias: region A on the Activation
    engine (Identity + bias AP; its act-table load is issued explicitly in
    the pre-barrier idle window) overlapping region B's last matmul;
    region B on DVE (tensor_scalar_add + bias AP).  Bias columns are raw
    f32 stored in 4 bf16 slots of chunk1.
  - Output: one pre-armed SWDGE scatter-add (bf16, 64KB/core) fired by a
    trigger when both evictions land; host transposes halves and upcasts.

Degrades to the previous-generation fp32 tiers if the v3 machinery fails
to build or run in the target environment.
"""

import numpy as np

N, C = 1024, 256
N_CORES = 8
ROWS = N // N_CORES  # 128
P = 128
W_PK = 770

_CACHE = {}


# ---------------------------------------------------------------- v3 tier --

def _build_v3():
    import concourse.bacc as bacc
    import concourse.mybir as mybir

    f32 = mybir.dt.float32
    bf16 = mybir.dt.bfloat16
    i16 = mybir.dt.int16
    nc = bacc.Bacc("TRN2", target_bir_lowering=False, debug=False,
                   num_devices=N_CORES)

    pk1_d = nc.dram_tensor("pk1", [P, 388], bf16, kind="ExternalInput").ap()
    pk2_d = nc.dram_tensor("pk2", [240, 384], bf16, kind="ExternalInput").ap()
    out_d = nc.dram_tensor("outb", [240, 256], bf16, kind="ExternalOutput").ap()

    insts = nc.main_func.blocks[0].instructions

    with (
        nc.sbuf_tensor([P, 388], bf16) as T1_t,
        nc.sbuf_tensor([P, 384], bf16) as T2_t,
        nc.sbuf_tensor([P, 256], bf16) as OT_t,
        nc.sbuf_tensor([P, 9], i16) as gg_t,
        nc.sbuf_tensor([P, 8], i16) as gi_t,
        nc.psum_tensor([P, 512], f32) as psA_t,
        nc.psum_tensor([P, 512], f32) as psB_t,
        nc.semaphore() as g_prep,
        nc.semaphore() as d1,
        nc.semaphore() as d2,
        nc.semaphore() as pa,
        nc.semaphore() as pb,
        nc.semaphore() as vv,
        nc.semaphore() as o_prep,
        nc.semaphore() as o_dma,
    ):
        T1 = T1_t.ap()
        T2 = T2_t.ap()
        OT = OT_t.ap()
        gg = gg_t.ap()
        gi = gi_t.ap()
        psA = psA_t.ap()[:, 0:128]
        psB = psB_t.ap()[:, 0:128]

        # ---- hoist group (runs before the Bass preamble barrier) ----
        n0 = len(insts)
        # gather idx: the ucode consumes the wrapped idx stream one
        # 16-entry vector ahead, so bake the +16 shift into the iota
        # (base -16, 9 columns); the never-consumed first column is negative
        nc.gpsimd.iota(gg[:], [[16, 9]], base=-16, channel_multiplier=1)
        nc.gpsimd.dma_gather(
            T2.rearrange("p (g m) -> p g m", g=1), pk2_d, gg[:],
            128, 128, 384, prepare_only=True, sem=d2).then_inc(g_prep, 1)
        nc.gpsimd.wait_ge(g_prep, 1)
        nc.gpsimd.trigger_dma(count=1)
        nc.sync.dma_start(T1[:], pk1_d[:]).then_inc(d1, 16)
        # preload the activation-function table in the Act engine's idle
        # pre-barrier window; insert_act_table_loads then sees Identity's
        # table loaded on every path and adds nothing before the eviction
        from concourse.hw_specs import get_activation_tables
        tables = get_activation_tables(nc.m.arch)
        set_id = next(i for i, funcs in enumerate(tables.values())
                      if mybir.ActivationFunctionType.Identity in funcs)
        ld = mybir.InstLoadActFuncSet(
            name=nc.get_next_instruction_name(), act_func_set_id=set_id,
            ins=[], outs=[])
        nc.scalar.add_instruction(ld)
        n1 = len(insts)

        # ---- PE: out^T in two psum banks; free dim = 128 rows ----
        nc.tensor.wait_ge(d1, 16)
        nc.tensor.matmul(psA, T1[:, 128:256], T1[:, 0:128],
                         start=True, stop=False)                    # A k=0:128
        nc.tensor.matmul(psB, T1[:, 256:384], T1[:, 0:128],
                         start=True, stop=False)                    # B k=0:128
        nc.tensor.wait_ge(d2, 16)
        nc.tensor.matmul(psA, T2[:, 128:256], T2[:, 0:128],
                         start=False, stop=True).then_inc(pa, 1)    # A k=128:256
        nc.tensor.matmul(psB, T2[:, 256:384], T2[:, 0:128],
                         start=False, stop=True).then_inc(pb, 1)    # B k=128:256

        # ---- evictions with fused per-partition bias (psum f32 -> sbuf
        # bf16): A on Activation (overlaps B's last matmul), B on DVE.
        # The bias columns are raw f32 values in 4 bf16 slots of T1. ----
        bias_f32 = T1[:, 384:388].bitcast(f32)          # [128, 2] f32
        nc.scalar.wait_ge(pa, 1)
        nc.scalar.activation(OT[:, 0:128], psA,
                             mybir.ActivationFunctionType.Identity,
                             bias=bias_f32[:, 0:1], scale=1.0).then_inc(vv, 1)
        nc.vector.wait_ge(pb, 1)
        nc.vector.tensor_scalar_add(OT[:, 128:256], psB,
                                    bias_f32[:, 1:2]).then_inc(vv, 1)

        # ---- Pool (post-preamble): arm + fire the output scatter ----
        nc.gpsimd.iota(gi[:], [[16, 8]], base=0, channel_multiplier=1)
        nc.gpsimd.dma_scatter_add(
            out_d, OT.rearrange("p (g m) -> p g m", g=1), gi[:],
            128, 128, 256, prepare_only=True, sem=o_dma).then_inc(o_prep, 1)
        nc.gpsimd.wait_ge(o_prep, 1)
        tout = nc.gpsimd.trigger_dma(count=1)
        tout._wait_ge(vv, 2)
        nc.sync.wait_ge(o_dma, 16)

        # move the hoist group ahead of the preamble's const memsets +
        # all-engine barrier, but after the per-engine register init
        # (RegisterMove/TPBBaseLd) so nothing runs on an uninitialized engine
        first_memset = next(i for i, inst in enumerate(insts)
                            if type(inst).__name__ == "InstMemset")
        moved = insts[n0:n1]
        for m in moved:
            insts.remove(m)
        for m in reversed(moved):
            insts.insert(first_memset, m)

    nc.compile()
    return nc


def _pack_v3(inputs):
    import ml_dtypes
    bf16 = ml_dtypes.bfloat16
    feat = np.asarray(inputs["feat"], dtype=np.float32)
    Wv = np.ascontiguousarray(np.asarray(inputs["Wv"], dtype=np.float32))
    bv = np.asarray(inputs["bv"], dtype=np.float32).reshape(C)

    Wvb = Wv.astype(bf16)
    maps = []
    for c in range(N_CORES):
        blk = feat[c * ROWS:(c + 1) * ROWS, :]          # [128 rows, 256 ch]
        ftT = blk.T.astype(bf16)                        # [256 ch, 128 rows]
        pk1 = np.zeros((P, 388), bf16)
        pk1[:, 0:128] = ftT[0:128, :]                   # ft1
        pk1[:, 128:256] = Wvb[0:128, 0:128]             # WvA1
        pk1[:, 256:384] = Wvb[0:128, 128:256]           # WvB1
        # bias columns: raw f32 values occupying 2 bf16 slots each
        pk1[:, 384:386] = bv[0:128].reshape(P, 1).view(bf16)    # biasA f32
        pk1[:, 386:388] = bv[128:256].reshape(P, 1).view(bf16)  # biasB f32
        pk2 = np.zeros((240, 384), bf16)
        pk2[0:P, 0:128] = ftT[128:256, :]               # ft2
        pk2[0:P, 128:256] = Wvb[128:256, 0:128]         # WvA2
        pk2[0:P, 256:384] = Wvb[128:256, 128:256]       # WvB2
        maps.append({"pk1": pk1, "pk2": pk2})
    return maps


def _unshard_v3(results):
    parts = []
    for c in range(N_CORES):
        ob = np.asarray(results[c]["outb"])[0:P].astype(np.float32)
        blk = np.empty((ROWS, C), np.float32)
        blk[:, 0:128] = ob[:, 0:128].T                  # out^T ch 0:128
        blk[:, 128:256] = ob[:, 128:256].T              # out^T ch 128:256
        parts.append(blk)
    return np.concatenate(parts, axis=0)


# ------------------------------------------------- legacy fp32 fallbacks --

def _build_fast():
    import concourse.bacc as bacc
    import concourse.mybir as mybir

    f32 = mybir.dt.float32
    i16 = mybir.dt.int16
    nc = bacc.Bacc("TRN2", target_bir_lowering=False, debug=False,
                   num_devices=N_CORES)

    pk_d = nc.dram_tensor("pk", [P, W_PK], f32, kind="ExternalInput").ap()
    out_d = nc.dram_tensor("outT", [C + P, ROWS], f32,
                           kind="ExternalOutput").ap()

    n_pre = len(nc.main_func.blocks[0].instructions)

    with (
        nc.sbuf_tensor([P, 256], f32) as A_t,
        nc.sbuf_tensor([P, 128], f32) as B_t,
        nc.sbuf_tensor([P, 256], f32) as C_t,
        nc.sbuf_tensor([P, 130], f32) as D_t,
        nc.sbuf_tensor([P, 2 * ROWS], f32) as ot_t,
        nc.sbuf_tensor([P, 16], i16) as idx_t,
        nc.psum_tensor([P, 512], f32) as psb0,
        nc.psum_tensor([P, 512], f32) as psb1,
        nc.semaphore() as d1,
        nc.semaphore() as dp,
        nc.semaphore() as d2,
        nc.semaphore() as d3,
        nc.semaphore() as pes,
        nc.semaphore() as v0,
        nc.semaphore() as v1,
        nc.semaphore() as prep_sem,
        nc.semaphore() as dout,
    ):
        A = A_t.ap()
        Bt = B_t.ap()
        Ct = C_t.ap()
        D = D_t.ap()
        ot = ot_t.ap()
        idx = idx_t.ap()
        ps0 = psb0.ap()[:, 0:ROWS]
        ps1 = psb1.ap()[:, 0:ROWS]

        nc.sync.dma_start(A[:], pk_d[:, 0:256]).then_inc(d1, 16)
        nc.sync.dma_start(Ct[:], pk_d[:, 384:640]).then_inc(d2, 16)
        nc.sync.dma_start(D[:], pk_d[:, 640:770]).then_inc(d3, 16)
        nc.gpsimd.dma_start(Bt[:], pk_d[:, 256:384]).then_inc(dp, 16)

        nc.gpsimd.iota(idx[:, 0:8], [[16, 8]], base=0, channel_multiplier=1)
        nc.gpsimd.iota(idx[:, 8:16], [[16, 8]], base=128, channel_multiplier=1)
        nc.gpsimd.dma_scatter_add(
            out_d[:, :], ot[:, 0:ROWS].rearrange("p (g m) -> p g m", g=1),
            idx[:, 0:8], ROWS, ROWS, ROWS,
            prepare_only=True, sem=dout).then_inc(prep_sem, 1)
        nc.gpsimd.dma_scatter_add(
            out_d[:, :], ot[:, ROWS:2 * ROWS].rearrange("p (g m) -> p g m", g=1),
            idx[:, 8:16], ROWS, ROWS, ROWS,
            prepare_only=True, sem=dout).then_inc(prep_sem, 1)

        nc.tensor.wait_ge(d1, 16)
        nc.tensor.matmul(ps0, A[:, 128:256], A[:, 0:128], start=True, stop=False)
        nc.tensor.wait_ge(dp, 16)
        nc.tensor.matmul(ps1, Bt[:, 0:128], A[:, 0:128], start=True, stop=False)
        nc.tensor.wait_ge(d2, 16)
        nc.tensor.matmul(ps0, Ct[:, 128:256], Ct[:, 0:128],
                         start=False, stop=True).then_inc(pes, 1)
        nc.tensor.wait_ge(d3, 16)
        nc.tensor.matmul(ps1, D[:, 0:128], Ct[:, 0:128],
                         start=False, stop=True).then_inc(pes, 1)

        nc.vector.wait_ge(pes, 1)
        nc.vector.tensor_scalar_add(
            ot[:, 0:ROWS], ps0, D[:, 128:129]).then_inc(v0, 1)
        nc.vector.wait_ge(pes, 2)
        nc.vector.tensor_scalar_add(
            ot[:, ROWS:2 * ROWS], ps1,
            D[:, 129:130]).then_inc(v1, 1)

        nc.gpsimd.wait_ge(prep_sem, 1)
        t0 = nc.gpsimd.trigger_dma(count=1)
        t0._wait_ge(v0, 1)
        nc.gpsimd.wait_ge(prep_sem, 2)
        t1 = nc.gpsimd.trigger_dma(count=1)
        t1._wait_ge(v1, 1)
        nc.sync.wait_ge(dout, 32)

        insts = nc.main_func.blocks[0].instructions
        moved = [i for i in insts[n_pre:]
                 if type(i).__name__ == "InstDMACopy"
                 and i.engine in (mybir.EngineType.SP,
                                  mybir.EngineType.Pool)][:4]
        moved += [i for i in insts[n_pre:]
                  if type(i).__name__ == "InstIota"][:2]
        for m in moved:
            insts.remove(m)
        for m in reversed(moved):
            insts.insert(0, m)

    nc.compile()
    return nc


def _build_fallback():
    """Plain Tile version: 2-way split packed input, 4 matmuls, DVE
    bias-add eviction, single output DMA, transposed output layout."""
    import concourse.bacc as bacc
    import concourse.bass as bass
    import concourse.mybir as mybir
    from concourse import tile

    f32 = mybir.dt.float32
    nc = bacc.Bacc("TRN2", target_bir_lowering=False, debug=False,
                   num_devices=N_CORES)

    pk_d = nc.dram_tensor("pk", [P, W_PK], f32, kind="ExternalInput").ap()
    out_d = nc.dram_tensor("outT", [C + P, ROWS], f32,
                           kind="ExternalOutput").ap()

    with tile.TileContext(nc) as tc:
        with (
            tc.tile_pool(name="sbuf", bufs=1) as pool,
            tc.tile_pool(name="psum", bufs=1, space=bass.MemorySpace.PSUM) as pp,
        ):
            pkA = pool.tile([P, 384], f32)
            pkB = pool.tile([P, 386], f32)
            ps0 = pp.tile([P, ROWS], f32, name="ps0")
            ps1 = pp.tile([P, ROWS], f32, name="ps1")
            ot = pool.tile([P, 2 * ROWS], f32)

            nc.sync.dma_start(pkA[:], pk_d[:, 0:384])
            nc.sync.dma_start(pkB[:], pk_d[:, 384:770])

            nc.tensor.matmul(ps0[:], pkA[:, 128:256], pkA[:, 0:128],
                             start=True, stop=False)
            nc.tensor.matmul(ps1[:], pkA[:, 256:384], pkA[:, 0:128],
                             start=True, stop=False)
            nc.tensor.matmul(ps0[:], pkB[:, 128:256], pkB[:, 0:128],
                             start=False, stop=True)
            nc.tensor.matmul(ps1[:], pkB[:, 256:384], pkB[:, 0:128],
                             start=False, stop=True)

            nc.vector.tensor_scalar_add(ot[:, 0:ROWS], ps0[:],
                                        pkB[:, 384:385])
            nc.vector.tensor_scalar_add(ot[:, ROWS:2 * ROWS], ps1[:],
                                        pkB[:, 385:386])

            nc.sync.dma_start(
                out_d[0:C].rearrange("(a p) m -> p a m", a=2),
                ot.rearrange("p (a m) -> p a m", a=2))

    nc.compile()
    return nc


def _pack_legacy(inputs):
    feat = np.asarray(inputs["feat"], dtype=np.float32)
    Wv = np.ascontiguousarray(np.asarray(inputs["Wv"], dtype=np.float32))
    bv = np.asarray(inputs["bv"], dtype=np.float32).reshape(C)
    bt = bv.reshape(2, P).T  # [P, 2]; col a holds bv[a*128 + p]
    maps = []
    for c in range(N_CORES):
        ftT = feat[c * ROWS:(c + 1) * ROWS, :].T  # [C, ROWS]
        pk = np.empty((P, W_PK), np.float32)
        pk[:, 0:128] = ftT[0:P, :]            # ftA
        pk[:, 128:256] = Wv[0:P, 0:128]       # wvA_a0
        pk[:, 256:384] = Wv[0:P, 128:256]     # wvA_a1
        pk[:, 384:512] = ftT[P:C, :]          # ftB
        pk[:, 512:640] = Wv[P:C, 0:128]       # wvB_a0
        pk[:, 640:768] = Wv[P:C, 128:256]     # wvB_a1
        pk[:, 768:770] = bt                   # bias
        maps.append({"pk": pk})
    return maps


def _unshard_legacy(results):
    parts = [np.ascontiguousarray(np.asarray(results[c]["outT"])[0:C].T)
             for c in range(N_CORES)]
    return np.concatenate(parts, axis=0)


_TIERS = [
    (_build_v3, _pack_v3, _unshard_v3),
    (_build_fast, _pack_legacy, _unshard_legacy),
    (_build_fallback, _pack_legacy, _unshard_legacy),
]


def _get_nc():
    if "nc" not in _CACHE:
        last = None
        for i in range(_CACHE.get("tier", 0), len(_TIERS)):
            try:
                _CACHE["nc"] = _TIERS[i][0]()
                _CACHE["tier"] = i
                break
            except Exception as e:
                last = e
        else:
            raise last
    return _CACHE["nc"]


def _run(inputs, **run_kwargs):
    from concourse.bass_utils import run_bass_kernel_spmd

    nc = _get_nc()
    tier = _CACHE["tier"]
    in_maps = _TIERS[tier][1](inputs)
    res = run_bass_kernel_spmd(nc, in_maps, list(range(N_CORES)), **run_kwargs)
    return _TIERS[tier][2](res.results), res


def kernel(**inputs) -> np.ndarray:
    while True:
        try:
            out, _ = _run(inputs)
            return out
        except Exception:
            # demote to the next, more conservative program tier and retry
            tier = _CACHE.get("tier", 0) + 1
            if "nc" not in _CACHE or tier >= len(_TIERS):
                raise
            _CACHE.pop("nc")
            _CACHE["tier"] = tier
